# revision 75
# baseline (speedup 1.0000x reference)
"""Cuboid (windowed) self-attention Trainium2 kernel.

x (2, 8, 64, 64, 256) -> 128 windows of (512 tokens, 256 ch); per window:
qkv = xw @ qkv_w, 8-head softmax attention (dh=32), proj + bias; reverse.

Sharding: data-parallel over the 128 windows across 8 NeuronCores
(16 windows/core), weights replicated.

Default config (DEFAULT_ATTN/QKV/SCHED): fp16 everywhere + "pvmb".
fp16 keeps the PE at full rate while avoiding f32r's 4-byte stationary
loads (fp16 QKV also measures 5x MORE accurate than the old
f32r-matmul-then-round-to-bf16: rel err 6.7e-4 vs 3.7e-3); all matmul
accumulation stays fp32 in PSUM. HW probing showed each matmul costs
its full serial ldweights+stream time (tile-position packing gives at
most ~1.25x), so the design minimizes total streamed columns:

  host pre-transposes each window to xT (256 ch, 512 tok), fp16
  QT/KT = Wq/Wk.T @ xT         (head-dim on partitions, tokens free)
  V     = xT.T @ Wv            (tokens on partitions, head-dim free),
                               scattered to vv[j, pair, eo] = [V_h|ones]
  S^T   = K @ Q^T              per head (keys on partitions), 2 heads
                               per PSUM region (contraction is 32)
  P^T   = exp(scale * S^T)     one ScalarE activation per (head-pair,
                               key-chunk), PSUM->SBUF fp16, 1024 wide
  [pvT|d] = [V_h|ones].T @ P^T ("pvm"): the M=64 stationary makes every
                               PV matmul also produce the softmax
                               denominator rows, deleting the 32
                               separate denominator matmuls (16384
                               streamed cols/window) the old design used
  normalize: d rows staged to SBUF (4 copies), ONE full-width
             reciprocal_approx_fast (the custom DVE op needs full-128-
             partition SBUF operands; it is ~5x faster than the exact
             6-cycles/element nc.vector.reciprocal), 4 muls into outT
  y = outT.T @ proj_w + b      (queries on partitions) -> DMA out

PSUM (8 banks): 2x [128,2,512] S regions ping-pong (4 banks) + 2x 4KB
rotating slots (4 banks) shared by the PV bank-pairs and all qkv/proj
accumulators (single tag so the FIFO rotation interleaves legally).

Emission is software-pipelined across windows (window w+1's QT/KT start
mid-window-w). "pvmb" = one extra buffer on every SBUF pool (pT pool
+2): the pT pool depth was the hidden serializer between ScalarE's exp
stream and the PV matmuls (~25% of the runtime).

DVE ops are width-merged where partition alignment allows (V-scatter
copies 4->2/window, tail bias adds 4->2/window via a [P,2,C] bias and
[P,2,C]-shaped tail accumulators) since the timeline sim shows
PE/ACT/DVE near-balanced (~17-19us/window each).

Measured (8-core SPMD, slope method): 555577 ns (session start,
bf16/f32r) -> ~240-260K ns, rel err 3.7e-3 -> 6.7e-4.
"""

import numpy as np

import concourse.bass as bass
import concourse.bacc as bacc
import concourse.tile as tile
from concourse import mybir
from concourse.bass_utils import run_bass_kernel_spmd

B, T, H, W, C = 2, 8, 64, 64, 256
HEADS = 8
WS = 8
DH = C // HEADS            # 32
N = T * WS * WS            # 512 tokens per window
NWIN = B * (H // WS) * (W // WS)   # 128
NCORES = 8
WPC = NWIN // NCORES       # 16 windows per core
SCALE = DH ** -0.5
P = 128

F32 = mybir.dt.float32
F32R = mybir.dt.float32r
BF16 = mybir.dt.bfloat16
F16 = mybir.dt.float16
EXP = mybir.ActivationFunctionType.Exp


def _emit(ctx, tc, nwin, adt, qdt, x_d, wqkv_d, wproj_d, bias_d, out_d,
          reps=1,
          variant="", sched="base"):
    nc = tc.nc

    pvm = sched in ("pvm", "gpvm", "pvmb", "pvmb2", "pvmb3", "pvmbs",
                    "pvmo", "pvmr")
    big = {"pvmb": 1, "pvmb2": 2, "pvmb3": 1, "pvmbs": 1, "pvmo": 1,
           "pvmr": 1}.get(sched, 0)
    xb = 1 if sched == "pvmb3" else 0   # extra pT/x/qk lookahead only
    singles = ctx.enter_context(tc.tile_pool(name="singles", bufs=1))
    xpool = ctx.enter_context(tc.tile_pool(name="xpool", bufs=2 + big + xb))
    qkpool = ctx.enter_context(tc.tile_pool(name="qkpool",
                                            bufs=2 + big + xb))
    ppool = ctx.enter_context(tc.tile_pool(name="ppool",
                                           bufs=3 + 2 * big + 3 * xb))
    opool = ctx.enter_context(tc.tile_pool(name="opool", bufs=2 + big))
    ypool = ctx.enter_context(tc.tile_pool(name="ypool", bufs=2 + big))
    mpool = ctx.enter_context(tc.tile_pool(name="mpool", bufs=2 + big))
    ps2 = ctx.enter_context(tc.tile_pool(name="ps2", bufs=2, space="PSUM"))
    ps1 = ctx.enter_context(tc.tile_pool(
        name="ps1",
        bufs=2 if pvm else 4,
        space="PSUM"))

    # ---- constants ----
    wqkv = singles.tile([P, 2, 3 * C], qdt)   # [:, cc, j]: channel chunk cc
    nc.sync.dma_start(wqkv, wqkv_d.rearrange("(g p) j -> p g j", p=P))
    wproj = singles.tile([P, 2, C], adt)
    if adt == qdt:
        nc.sync.dma_start(wproj, wproj_d.rearrange("(g p) j -> p g j", p=P))
    else:
        wproj_f = singles.tile([P, 2, C], qdt)
        nc.sync.dma_start(wproj_f, wproj_d.rearrange("(g p) j -> p g j", p=P))
        nc.vector.tensor_copy(wproj, wproj_f)
    bias = singles.tile([P, 2, C], F32)
    for k in range(2):
        nc.sync.dma_start(
            bias[:, k, :],
            bass.AP(tensor=bias_d.tensor, offset=bias_d.offset,
                    ap=[[0, P], [1, C]]),
        )
    ones = singles.tile([P, 32], adt)
    nc.vector.memset(ones, 1.0)

    def cd_of(hh):
        return (32 * hh + 64) % 128

    st = [None] * nwin   # per-window live tiles

    # probe variants: static PSUM pair for "nos" (exp reads these instead
    # of per-step matmul output)
    if variant == "nos":
        sp_static = [ps2.tile([P, 2, N], F32, tag="s", name=f"spst{i}")
                     for i in range(2)]
        nc.vector.memset(sp_static[0], 0.25)
        nc.vector.memset(sp_static[1], 0.5)

    def pj_of(step):
        # g4: j-major within a head-group, so consecutive s_steps cover
        # all 4 PE row groups (rows 0/32 for even pair, 64/96 for odd)
        # and 4-way row-tile packing can engage.
        if sched == "g4":
            return step % 2 + 2 * (step // 8), (step // 2) % 4
        return step // 4, step % 4

    def start_window(w):
        s = {"s": {}, "p": {}, "pv": {}, "d": {}}
        s["xt"] = xpool.tile([P, 2, N], qdt, tag="xt", name="xt")
        nc.sync.dma_start(s["xt"], x_d[w].rearrange("(g p) n -> p g n", p=P))
        s["qt"] = qkpool.tile([P, 2, N], adt, tag="qt", name="qt")
        s["kt"] = qkpool.tile([P, 2, N], adt, tag="kt", name="kt")
        if pvm:
            # vv[:, j, pair, 0] = [V_h_even | ones], [.., 1] = [ones |
            # V_h_odd]: the PV matmul stationary (M=64) then also produces
            # the softmax denominator rows in the same matmul, so the 32
            # separate denominator matmuls disappear.
            s["vv"] = qkpool.tile([P, 4, 4, 2, 64], adt, tag="vv", name="vv")
            if sched != "pvmo" or w < 2 + big:
                # pool slots rotate but the ones columns are never
                # overwritten, so once every slot has been set the memset
                # (and its WAR edge on the prior window's PV reads) can
                # be skipped
                nc.vector.memset(s["vv"][:, :, :, :, 32:64], 1.0)
        else:
            s["vv"] = qkpool.tile([P, 2, N], adt, tag="vv", name="vv")
        s["outT"] = opool.tile([P, 2, N], adt, tag="outT", name="outT")
        if variant == "noqkv":
            nc.vector.memset(s["qt"], 0.1)
            nc.vector.memset(s["kt"], 0.1)
            nc.vector.memset(s["vv"], 0.1)
        if variant == "nopvd":
            nc.vector.memset(s["outT"], 0.1)
        st[w] = s

    def acc1():
        # pvm: all ps1 slots share one 4KB tag so the PV bank double-
        # buffers inside the same 4-bank budget; accs use half a slot.
        if pvm:
            return ps1.tile([P, 2, N], F32, tag="bank", name="acc")[:, 0, :]
        return ps1.tile([P, N], F32, tag="b1", name="acc")

    def qkv_qk(w, g):
        # Q^T,K^T head-group g: out = W[:, cols].T @ xT
        if variant == "noqkv":
            return
        s = st[w]
        for name, base in (("qt", 0), ("kt", 256)):
            acc = acc1()
            for cc in range(2):
                nc.tensor.matmul(
                    acc,
                    lhsT=wqkv[:, cc, base + 128 * g:base + 128 * (g + 1)],
                    rhs=s["xt"][:, cc, :],
                    start=(cc == 0), stop=(cc == 1))
            nc.vector.tensor_copy(s[name][:, g, :], acc)

    def qkv_v(w):
        # V: out = xT.T @ Wv   (tokens on partitions)
        if variant == "noqkv":
            return
        s = st[w]
        for half in range(2):
            if pvm:
                acc = ps1.tile([P, 2, 2, 4, 2, 32], F32, tag="bank",
                               name="acc")[:, 0]
                for sub in range(2):     # token chunk m = 2*half + sub
                    m = 2 * half + sub
                    for cc in range(2):
                        nc.tensor.matmul(
                            acc[:, sub],
                            lhsT=s["xt"][:, cc, 128 * m:128 * (m + 1)],
                            rhs=wqkv[:, cc, 512:768],
                            start=(cc == 0), stop=(cc == 1))
                nc.vector.tensor_copy(
                    s["vv"][:, 2 * half:2 * half + 2, :, :, 0:32], acc)
                continue
            acc = ps1.tile([P, N], F32, tag="b1", name="acc")
            for sub in range(2):     # token chunk m = 2*half + sub
                m = 2 * half + sub
                for cc in range(2):
                    nc.tensor.matmul(
                        acc[:, sub * C:(sub + 1) * C],
                        lhsT=s["xt"][:, cc, 128 * m:128 * (m + 1)],
                        rhs=wqkv[:, cc, 512:768],
                        start=(cc == 0), stop=(cc == 1))
            nc.vector.tensor_copy(s["vv"][:, half, :], acc)

    def s_step(w, step):
        # S^T for (head pair, key chunk): keys on partitions, queries free
        s = st[w]
        if variant == "nos":
            s["s"][step] = sp_static[step % 2]
            return
        pair, j = pj_of(step)
        g = pair // 2
        sp = ps2.tile([P, 2, N], F32, tag="s", name="sp")
        for hl in range(2):
            row = 64 * (pair % 2) + 32 * hl
            nc.tensor.matmul(
                sp[:, hl, :],
                lhsT=s["kt"][row:row + 32, g, 128 * j:128 * (j + 1)],
                rhs=s["qt"][row:row + 32, g, :],
                start=True, stop=True,
                tile_position=(row, 0))
        s["s"][step] = sp

    def exp_step(w, step):
        s = st[w]
        pT = ppool.tile([P, 2, N], adt, tag="pT", name="pT")
        if variant == "exphalf":
            # probe: half the ACT work (other half of pT left stale)
            nc.scalar.activation(pT[:, 0, :], s["s"].pop(step)[:, 0, :],
                                 EXP, scale=SCALE)
        else:
            nc.scalar.activation(pT, s["s"].pop(step), EXP, scale=SCALE)
        s["p"][step] = pT

    def finish_gpair(w, g):
        # pvm: bankpair [128, 2 pairs, 512]: rows [pv_e|d_e|pv_o|d_o] per
        # pair. Stage the d rows in SBUF (approx recip mis-streams on
        # PSUM), reciprocal once per row-block across both pairs, then 4
        # muls into outT.
        s = st[w]
        bankpair = s["pv"].pop(g)
        sbd = mpool.tile([P, N], F32, tag="sbd", name="sbd")
        rc = mpool.tile([P, N], F32, tag="rc", name="rc")
        dcopy = nc.scalar.copy if sched == "pvmbs" else nc.vector.tensor_copy
        for b in range(2):
            for eo in range(2):
                hh = 2 * b + eo
                dcopy(
                    sbd[32 * hh:32 * hh + 32],
                    bankpair[64 * eo + 32:64 * eo + 64, b, :])
        nc.vector.reciprocal_approx_fast(rc, sbd)
        for b in range(2):
            for eo in range(2):
                hh = 2 * b + eo
                nc.vector.tensor_mul(
                    s["outT"][32 * hh:32 * hh + 32, g, :],
                    bankpair[64 * eo:64 * eo + 32, b, :],
                    rc[32 * hh:32 * hh + 32])

    def pvd_step(w, step):
        # col-packed: 2 PV matmuls + 2 denominator matmuls, accum over j
        s = st[w]
        if variant == "nopvd":
            s["p"].pop(step)
            return
        pair, j = pj_of(step)
        if pvm:
            g = pair // 2
            if j == 0 and pair % 2 == 0:
                s["pv"][g] = ps1.tile([P, 2, N], F32, tag="bank",
                                      name="bank")
            bank = s["pv"][g][:, pair % 2, :]
            pT = s["p"].pop(step)
            for eo in range(2):
                nc.tensor.matmul(
                    bank[64 * eo:64 * eo + 64, :],
                    lhsT=s["vv"][:, j, pair, eo, :],
                    rhs=pT[:, eo, :],
                    start=(j == 0), stop=(j == 3),
                    tile_position=(0, 64 * eo), skip_group_check=True)
            if j == 3 and pair % 2 == 1:
                finish_gpair(w, g)
            return
        g = pair // 2
        if step % 8 == 0:
            s["pv"][g] = ps1.tile([P, N], F32, tag="b1", name="pv")
            s["d"][g] = ps1.tile([P, N], F32, tag="b1", name="d")
        pT = s["p"].pop(step)
        for hl in range(2):
            h = 2 * pair + hl
            hh = h % 4
            nc.tensor.matmul(
                s["pv"][g][32 * hh:32 * hh + 32, :],
                lhsT=s["vv"][:, j // 2, (j % 2) * C + 32 * h:(j % 2) * C + 32 * h + 32],
                rhs=pT[:, hl, :],
                start=(j == 0), stop=(j == 3),
                tile_position=(0, 32 * hh), skip_group_check=True)
            if variant == "nodenom" and j > 0:
                continue   # probe: 1 denom matmul per (head) instead of 4
            nc.tensor.matmul(
                s["d"][g][cd_of(hh):cd_of(hh) + 32, :],
                lhsT=ones,
                rhs=pT[:, hl, :],
                start=(j == 0), stop=(j == 3 or variant == "nodenom"),
                tile_position=(0, cd_of(hh)), skip_group_check=True)

    def finish_g(w, g):
        # 1/denom, un-rotating the +64-partition shift the D col-groups use
        # (D lands at (32*hh+64)%128 so it can pack with PV in the PE array)
        if variant == "nopvd" or pvm:
            return
        s = st[w]
        d = s["d"].pop(g)
        rc = mpool.tile([P, N], F32, tag="rc", name="rc")
        sbd = mpool.tile([P, N], F32, tag="sbd", name="sbd")
        # the approx reciprocal only streams correctly on full-128-
        # partition SBUF operands: un-rotate d during the PSUM->SBUF
        # copies, then one full-width approx
        nc.vector.tensor_copy(sbd[0:64], d[64:128])
        nc.vector.tensor_copy(sbd[64:128], d[0:64])
        nc.vector.reciprocal_approx_fast(rc, sbd)
        nc.vector.tensor_mul(s["outT"][:, g, :], s["pv"].pop(g), rc)

    def tail(w):
        s = st[w]
        y = ypool.tile([P, 4, C], F32, tag="y")
        if variant == "notail":
            nc.vector.memset(y[:, 0, :], 0.0)
            nc.sync.dma_start(out_d[w].rearrange("(m p) c -> p m c", p=P), y)
            st[w] = None
            return
        for half in range(2):
            if pvm:
                acc = ps1.tile([P, 2, 2, C], F32, tag="bank",
                               name="acc")[:, 0]
            else:
                acc = acc1()
            for sub in range(2):
                m = 2 * half + sub
                dst = acc[:, sub] if pvm else acc[:, sub * C:(sub + 1) * C]
                for g in range(2):
                    nc.tensor.matmul(
                        dst,
                        lhsT=s["outT"][:, g, 128 * m:128 * (m + 1)],
                        rhs=wproj[:, g, :],
                        start=(g == 0), stop=(g == 1))
            if pvm:
                nc.vector.tensor_add(y[:, 2 * half:2 * half + 2, :],
                                     acc, bias)
            else:
                for sub in range(2):
                    nc.vector.tensor_add(y[:, 2 * half + sub, :],
                                         acc[:, sub * C:(sub + 1) * C],
                                         bias[:, 0, :])
        nc.sync.dma_start(out_d[w].rearrange("(m p) c -> p m c", p=P), y)
        st[w] = None

    # ---- pipelined emission ----
    def one_pass_grouped():
        # Same dataflow as one_pass, but PE matmuls are emitted in
        # same-tiling-mode batches (row-tiled S pairs, then col-tiled PVD
        # pairs) to halve PE array mode switches (each switch drains the
        # array) and put packed matmuls back-to-back so tile-position
        # concurrency can engage.
        start_window(0)
        qkv_qk(0, 0)
        qkv_qk(0, 1)
        qkv_v(0)
        s_step(0, 0)
        s_step(0, 1)
        for w in range(nwin):
            for Dd in range(8):
                t = 2 * Dd
                exp_step(w, t)
                exp_step(w, t + 1)
                for u in (t + 2, t + 3):
                    if u < 16:
                        s_step(w, u)
                    elif w + 1 < nwin:
                        s_step(w + 1, u - 16)
                pvd_step(w, t)
                pvd_step(w, t + 1)
                if Dd == 3:
                    finish_g(w, 0)
                    if w + 1 < nwin:
                        start_window(w + 1)
                        qkv_qk(w + 1, 0)
            finish_g(w, 1)
            if w + 1 < nwin:
                qkv_v(w + 1)
                qkv_qk(w + 1, 1)
            tail(w)

    def s_only_pass(four):
        # probe: ONLY the S matmuls, 2-way vs 4-way row-packed, nothing
        # else on PE. Discriminates serial vs concurrent packed matmuls.
        for w in range(nwin):
            qt = qkpool.tile([P, 2, N], adt, tag="qt", name="qt")
            kt = qkpool.tile([P, 2, N], adt, tag="kt", name="kt")
            nc.vector.memset(qt, 0.1)
            nc.vector.memset(kt, 0.1)
            if four:
                for g in range(2):
                    for j in range(4):
                        spA = ps2.tile([P, 2, N], F32, tag="s", name="spA")
                        spB = ps2.tile([P, 2, N], F32, tag="s", name="spB")
                        for hh in range(4):
                            sp = spA if hh < 2 else spB
                            nc.tensor.matmul(
                                sp[:, hh % 2, :],
                                lhsT=kt[32 * hh:32 * hh + 32, g,
                                        128 * j:128 * (j + 1)],
                                rhs=qt[32 * hh:32 * hh + 32, g, :],
                                start=True, stop=True,
                                tile_position=(32 * hh, 0))
            else:
                for step in range(16):
                    pair, j = step // 4, step % 4
                    g = pair // 2
                    sp = ps2.tile([P, 2, N], F32, tag="s", name="sp")
                    for hl in range(2):
                        row = 64 * (pair % 2) + 32 * hl
                        nc.tensor.matmul(
                            sp[:, hl, :],
                            lhsT=kt[row:row + 32, g, 128 * j:128 * (j + 1)],
                            rhs=qt[row:row + 32, g, :],
                            start=True, stop=True,
                            tile_position=(row, 0))
        y = ypool.tile([P, 4, C], F32, tag="y")
        nc.vector.memset(y, 0.0)
        for w in range(nwin):
            nc.sync.dma_start(out_d[w].rearrange("(m p) c -> p m c", p=P), y)

    def act_only_pass():
        # probe: pure ScalarE stream — 16 activations/window off a fixed
        # PSUM pair, no matmuls. Calibrates peak ACT throughput on HW.
        # variant "actmm" adds an independent PE matmul stream into other
        # PSUM banks to measure ACT/PE PSUM-port contention.
        spA = ps2.tile([P, 2, N], F32, tag="s", name="spA")
        spB = ps2.tile([P, 2, N], F32, tag="s", name="spB")
        nc.vector.memset(spA, 0.25)
        nc.vector.memset(spB, 0.5)
        if variant == "actmm":
            qt = qkpool.tile([P, 2, N], adt, tag="qt", name="qt")
            kt = qkpool.tile([P, 2, N], adt, tag="kt", name="kt")
            nc.vector.memset(qt, 0.1)
            nc.vector.memset(kt, 0.1)
        for w in range(nwin):
            for step in range(16):
                pT = ppool.tile([P, 2, N], adt, tag="pT", name="pT")
                nc.scalar.activation(pT, spA if step % 2 == 0 else spB,
                                     EXP, scale=SCALE)
                if variant == "actmm":
                    mmout = ps1.tile([P, N], F32, tag="b1", name="mmout")
                    for hl in range(2):
                        nc.tensor.matmul(
                            mmout, lhsT=kt[32 * hl:32 * hl + 32, 0, 0:128],
                            rhs=qt[32 * hl:32 * hl + 32, 0, :],
                            start=(hl == 0), stop=(hl == 1),
                            tile_position=(32 * hl, 0),
                            skip_group_check=True)
        # keep the output tensor written so the harness contract holds
        y = ypool.tile([P, 4, C], F32, tag="y")
        nc.vector.memset(y, 0.0)
        for w in range(nwin):
            nc.sync.dma_start(out_d[w].rearrange("(m p) c -> p m c", p=P), y)

    def one_pass():
        start_window(0)
        qkv_qk(0, 0)
        qkv_qk(0, 1)
        qkv_v(0)
        s_step(0, 0)
        s_step(0, 1)
        for w in range(nwin):
            for step in range(16):
                exp_step(w, step)
                t = step + 2
                if sched == "pvmr":
                    # PE queue: pvd (longer downstream chain) ahead of
                    # the next s-burst; both gate on exp(step)
                    pvd_step(w, step)
                if t < 16:
                    s_step(w, t)
                elif w + 1 < nwin:
                    s_step(w + 1, t - 16)
                if sched != "pvmr":
                    pvd_step(w, step)
                if step == 7:
                    finish_g(w, 0)
                    if w + 1 < nwin:
                        start_window(w + 1)
                        qkv_qk(w + 1, 0)
            finish_g(w, 1)
            if w + 1 < nwin:
                qkv_v(w + 1)
                qkv_qk(w + 1, 1)
            tail(w)

    if variant in ("actonly", "actmm"):
        body = act_only_pass
    elif variant in ("sonly", "sonly4"):
        def body():
            s_only_pass(variant == "sonly4")
    elif sched in ("grouped", "gpvm", "g4"):
        body = one_pass_grouped
    else:
        body = one_pass
    if reps == 1:
        body()
    else:
        # device-side repeat loop, for timing: isolates kernel time from
        # the ~90ms axon dispatch overhead
        with tc.For_i(0, reps, 1):
            body()


def _build_bass(nwin: int, adt, qdt, reps: int = 1, variant: str = "",
                sched: str = "base") -> bass.Bass:
    nc = bacc.Bacc("TRN2", target_bir_lowering=False)
    x_d = nc.declare_dram_parameter("xt", [nwin, C, N], qdt, isOutput=False)
    wqkv_d = nc.declare_dram_parameter("qkv_w", [C, 3 * C], qdt, isOutput=False)
    wproj_d = nc.declare_dram_parameter("proj_w", [C, C], qdt, isOutput=False)
    bias_d = nc.declare_dram_parameter("proj_b", [C], F32, isOutput=False)
    out_d = nc.declare_dram_parameter("out", [nwin, N, C], F32, isOutput=True)
    from contextlib import ExitStack
    with tile.TileContext(nc) as tc, ExitStack() as ctx:
        _emit(ctx, tc, nwin, adt, qdt, x_d.ap(), wqkv_d.ap(), wproj_d.ap(),
              bias_d.ap(), out_d.ap(), reps=reps, variant=variant,
              sched=sched)
    nc.compile()
    return nc


_CACHE: dict = {}
DTS = {"bf16": BF16, "f32r": F32R, "f32": F32, "f16": F16}

DEFAULT_ATTN = "f16"
DEFAULT_QKV = "f16"
DEFAULT_SCHED = "pvmo"


def get_nc(nwin=WPC, attn=None, qkv=None, reps=1, variant="", sched=None):
    attn = attn or DEFAULT_ATTN
    qkv = qkv or DEFAULT_QKV
    sched = sched or DEFAULT_SCHED
    key = (nwin, attn, qkv, reps, variant, sched)
    if key not in _CACHE:
        _CACHE[key] = _build_bass(nwin, DTS[attn], DTS[qkv], reps=reps,
                                  variant=variant, sched=sched)
    return _CACHE[key]


def shard_inputs(x, qkv_w, proj_w, proj_b, wpc=WPC, qkv=None):
    qkv = qkv or DEFAULT_QKV
    npdt = np.float16 if qkv == "f16" else np.float32
    hn, wn = H // WS, W // WS
    xw = np.asarray(x, dtype=np.float32).reshape(B, T, hn, WS, wn, WS, C)
    xw = xw.transpose(0, 2, 4, 1, 3, 5, 6).reshape(NWIN, N, C)
    xT = np.ascontiguousarray(xw.transpose(0, 2, 1).astype(npdt))
    return [
        {
            "xt": xT[i * wpc:(i + 1) * wpc],
            "qkv_w": np.asarray(qkv_w, dtype=npdt),
            "proj_w": np.asarray(proj_w, dtype=npdt),
            "proj_b": np.asarray(proj_b, dtype=np.float32),
        }
        for i in range(NCORES)
    ]


def unshard(results):
    y = np.concatenate([np.asarray(results[i]["out"]) for i in range(NCORES)],
                       axis=0)
    hn, wn = H // WS, W // WS
    y = y.reshape(B, hn, wn, T, WS, WS, C)
    y = y.transpose(0, 3, 1, 4, 2, 5, 6).reshape(B, T, H, W, C)
    return np.ascontiguousarray(y.astype(np.float32))


def kernel(x, qkv_w, proj_w, proj_b):
    nc = get_nc()
    in_maps = shard_inputs(x, qkv_w, proj_w, proj_b)
    res = run_bass_kernel_spmd(nc, in_maps, list(range(NCORES))).results
    return unshard(res)


if __name__ == "__main__":
    rng = np.random.default_rng(0)
    x = rng.standard_normal((B, T, H, W, C), dtype=np.float32)
    qkv_w = (rng.standard_normal((C, 3 * C), dtype=np.float32) * C ** -0.5)
    proj_w = (rng.standard_normal((C, C), dtype=np.float32) * C ** -0.5)
    proj_b = np.zeros((C,), dtype=np.float32)
    y = kernel(x, qkv_w, proj_w, proj_b)
    print(y.shape, y.dtype)



# revision 78
# speedup vs baseline: 1.2958x; 1.2958x over previous
"""Cuboid (windowed) self-attention Trainium2 kernel.

x (2, 8, 64, 64, 256) -> 128 windows of (512 tokens, 256 ch); per window:
qkv = xw @ qkv_w, 8-head softmax attention (dh=32), proj + bias; reverse.

Sharding: data-parallel over the 128 windows across 8 NeuronCores
(16 windows/core), weights replicated.

Default config (DEFAULT_ATTN/QKV/SCHED): fp16 everywhere + "pvmb".
fp16 keeps the PE at full rate while avoiding f32r's 4-byte stationary
loads (fp16 QKV also measures 5x MORE accurate than the old
f32r-matmul-then-round-to-bf16: rel err 6.7e-4 vs 3.7e-3); all matmul
accumulation stays fp32 in PSUM. HW probing showed each matmul costs
its full serial ldweights+stream time (tile-position packing gives at
most ~1.25x), so the design minimizes total streamed columns:

  host pre-transposes each window to xT (256 ch, 512 tok), fp16
  QT/KT = Wq/Wk.T @ xT         (head-dim on partitions, tokens free)
  V     = xT.T @ Wv            (tokens on partitions, head-dim free),
                               scattered to vv[j, pair, eo] = [V_h|ones]
  S^T   = K @ Q^T              per head (keys on partitions), 2 heads
                               per PSUM region (contraction is 32)
  P^T   = exp(scale * S^T)     one ScalarE activation per (head-pair,
                               key-chunk), PSUM->SBUF fp16, 1024 wide
  [pvT|d] = [V_h|ones].T @ P^T ("pvm"): the M=64 stationary makes every
                               PV matmul also produce the softmax
                               denominator rows, deleting the 32
                               separate denominator matmuls (16384
                               streamed cols/window) the old design used
  normalize: d rows staged to SBUF (4 copies), ONE full-width
             reciprocal_approx_fast (the custom DVE op needs full-128-
             partition SBUF operands; it is ~5x faster than the exact
             6-cycles/element nc.vector.reciprocal), 4 muls into outT
  y = outT.T @ proj_w + b      (queries on partitions) -> DMA out

PSUM (8 banks): 2x [128,2,512] S regions ping-pong (4 banks) + 2x 4KB
rotating slots (4 banks) shared by the PV bank-pairs and all qkv/proj
accumulators (single tag so the FIFO rotation interleaves legally).

Emission is software-pipelined across windows (window w+1's QT/KT start
mid-window-w). "pvmb" = one extra buffer on every SBUF pool (pT pool
+2): the pT pool depth was the hidden serializer between ScalarE's exp
stream and the PV matmuls (~25% of the runtime).

DVE ops are width-merged where partition alignment allows (V-scatter
copies 4->2/window, tail bias adds 4->2/window via a [P,2,C] bias and
[P,2,C]-shaped tail accumulators) since the timeline sim shows
PE/ACT/DVE near-balanced (~17-19us/window each).

Measured (8-core SPMD, slope method): 555577 ns (session start,
bf16/f32r) -> ~240-260K ns, rel err 3.7e-3 -> 6.7e-4.
"""

import numpy as np

import concourse.bass as bass
import concourse.bacc as bacc
import concourse.tile as tile
from concourse import mybir
from concourse.bass_utils import run_bass_kernel_spmd

B, T, H, W, C = 2, 8, 64, 64, 256
HEADS = 8
WS = 8
DH = C // HEADS            # 32
N = T * WS * WS            # 512 tokens per window
NWIN = B * (H // WS) * (W // WS)   # 128
NCORES = 8
WPC = NWIN // NCORES       # 16 windows per core
SCALE = DH ** -0.5
P = 128

F32 = mybir.dt.float32
F32R = mybir.dt.float32r
BF16 = mybir.dt.bfloat16
F16 = mybir.dt.float16
EXP = mybir.ActivationFunctionType.Exp


def _emit(ctx, tc, nwin, adt, qdt, x_d, wqkv_d, wproj_d, bias_d, out_d,
          reps=1,
          variant="", sched="base"):
    nc = tc.nc

    pvm = sched in ("pvm", "gpvm", "pvmb", "pvmb2", "pvmb3", "pvmbs",
                    "pvmo", "pvmr", "pvmor")
    big = {"pvmb": 1, "pvmb2": 2, "pvmb3": 1, "pvmbs": 1, "pvmo": 1,
           "pvmr": 1, "pvmor": 1}.get(sched, 0)
    xb = 1 if sched == "pvmb3" else 0   # extra pT/x/qk lookahead only
    singles = ctx.enter_context(tc.tile_pool(name="singles", bufs=1))
    xpool = ctx.enter_context(tc.tile_pool(name="xpool", bufs=2 + big + xb))
    qkpool = ctx.enter_context(tc.tile_pool(name="qkpool",
                                            bufs=2 + big + xb))
    ppool = ctx.enter_context(tc.tile_pool(name="ppool",
                                           bufs=3 + 2 * big + 3 * xb))
    opool = ctx.enter_context(tc.tile_pool(name="opool", bufs=2 + big))
    ypool = ctx.enter_context(tc.tile_pool(name="ypool", bufs=2 + big))
    mpool = ctx.enter_context(tc.tile_pool(name="mpool", bufs=2 + big))
    ps2 = ctx.enter_context(tc.tile_pool(name="ps2", bufs=2, space="PSUM"))
    ps1 = ctx.enter_context(tc.tile_pool(
        name="ps1",
        bufs=2 if pvm else 4,
        space="PSUM"))

    # ---- constants ----
    wqkv = singles.tile([P, 2, 3 * C], qdt)   # [:, cc, j]: channel chunk cc
    nc.sync.dma_start(wqkv, wqkv_d.rearrange("(g p) j -> p g j", p=P))
    wproj = singles.tile([P, 2, C], adt)
    if adt == qdt:
        nc.sync.dma_start(wproj, wproj_d.rearrange("(g p) j -> p g j", p=P))
    else:
        wproj_f = singles.tile([P, 2, C], qdt)
        nc.sync.dma_start(wproj_f, wproj_d.rearrange("(g p) j -> p g j", p=P))
        nc.vector.tensor_copy(wproj, wproj_f)
    bias = singles.tile([P, 2, C], F32)
    for k in range(2):
        nc.sync.dma_start(
            bias[:, k, :],
            bass.AP(tensor=bias_d.tensor, offset=bias_d.offset,
                    ap=[[0, P], [1, C]]),
        )
    ones = singles.tile([P, 32], adt)
    nc.vector.memset(ones, 1.0)

    def cd_of(hh):
        return (32 * hh + 64) % 128

    st = [None] * nwin   # per-window live tiles

    # probe variants: static PSUM pair for "nos" (exp reads these instead
    # of per-step matmul output)
    if variant == "nos":
        sp_static = [ps2.tile([P, 2, N], F32, tag="s", name=f"spst{i}")
                     for i in range(2)]
        nc.vector.memset(sp_static[0], 0.25)
        nc.vector.memset(sp_static[1], 0.5)

    def pj_of(step):
        # g4: j-major within a head-group, so consecutive s_steps cover
        # all 4 PE row groups (rows 0/32 for even pair, 64/96 for odd)
        # and 4-way row-tile packing can engage.
        if sched == "g4":
            return step % 2 + 2 * (step // 8), (step // 2) % 4
        return step // 4, step % 4

    def start_window(w):
        s = {"s": {}, "p": {}, "pv": {}, "d": {}}
        s["xt"] = xpool.tile([P, 2, N], qdt, tag="xt", name="xt")
        nc.sync.dma_start(s["xt"], x_d[w].rearrange("(g p) n -> p g n", p=P))
        s["qt"] = qkpool.tile([P, 2, N], adt, tag="qt", name="qt")
        s["kt"] = qkpool.tile([P, 2, N], adt, tag="kt", name="kt")
        if pvm:
            # vv[:, j, pair, 0] = [V_h_even | ones], [.., 1] = [ones |
            # V_h_odd]: the PV matmul stationary (M=64) then also produces
            # the softmax denominator rows in the same matmul, so the 32
            # separate denominator matmuls disappear.
            s["vv"] = qkpool.tile([P, 4, 4, 2, 64], adt, tag="vv", name="vv")
            if sched not in ("pvmo", "pvmor") or w < 2 + big:
                # pool slots rotate but the ones columns are never
                # overwritten, so once every slot has been set the memset
                # (and its WAR edge on the prior window's PV reads) can
                # be skipped
                nc.vector.memset(s["vv"][:, :, :, :, 32:64], 1.0)
        else:
            s["vv"] = qkpool.tile([P, 2, N], adt, tag="vv", name="vv")
        s["outT"] = opool.tile([P, 2, N], adt, tag="outT", name="outT")
        if variant == "noqkv":
            nc.vector.memset(s["qt"], 0.1)
            nc.vector.memset(s["kt"], 0.1)
            nc.vector.memset(s["vv"], 0.1)
        if variant == "nopvd":
            nc.vector.memset(s["outT"], 0.1)
        st[w] = s

    def acc1():
        # pvm: all ps1 slots share one 4KB tag so the PV bank double-
        # buffers inside the same 4-bank budget; accs use half a slot.
        if pvm:
            return ps1.tile([P, 2, N], F32, tag="bank", name="acc")[:, 0, :]
        return ps1.tile([P, N], F32, tag="b1", name="acc")

    def qkv_qk(w, g):
        # Q^T,K^T head-group g: out = W[:, cols].T @ xT
        if variant == "noqkv":
            return
        s = st[w]
        for name, base in (("qt", 0), ("kt", 256)):
            acc = acc1()
            for cc in range(2):
                nc.tensor.matmul(
                    acc,
                    lhsT=wqkv[:, cc, base + 128 * g:base + 128 * (g + 1)],
                    rhs=s["xt"][:, cc, :],
                    start=(cc == 0), stop=(cc == 1))
            nc.vector.tensor_copy(s[name][:, g, :], acc)

    def qkv_v(w):
        # V: out = xT.T @ Wv   (tokens on partitions)
        if variant == "noqkv":
            return
        s = st[w]
        for half in range(2):
            if pvm:
                acc = ps1.tile([P, 2, 2, 4, 2, 32], F32, tag="bank",
                               name="acc")[:, 0]
                for sub in range(2):     # token chunk m = 2*half + sub
                    m = 2 * half + sub
                    for cc in range(2):
                        nc.tensor.matmul(
                            acc[:, sub],
                            lhsT=s["xt"][:, cc, 128 * m:128 * (m + 1)],
                            rhs=wqkv[:, cc, 512:768],
                            start=(cc == 0), stop=(cc == 1))
                nc.vector.tensor_copy(
                    s["vv"][:, 2 * half:2 * half + 2, :, :, 0:32], acc)
                continue
            acc = ps1.tile([P, N], F32, tag="b1", name="acc")
            for sub in range(2):     # token chunk m = 2*half + sub
                m = 2 * half + sub
                for cc in range(2):
                    nc.tensor.matmul(
                        acc[:, sub * C:(sub + 1) * C],
                        lhsT=s["xt"][:, cc, 128 * m:128 * (m + 1)],
                        rhs=wqkv[:, cc, 512:768],
                        start=(cc == 0), stop=(cc == 1))
            nc.vector.tensor_copy(s["vv"][:, half, :], acc)

    def s_step(w, step):
        # S^T for (head pair, key chunk): keys on partitions, queries free
        s = st[w]
        if variant == "nos":
            s["s"][step] = sp_static[step % 2]
            return
        pair, j = pj_of(step)
        g = pair // 2
        sp = ps2.tile([P, 2, N], F32, tag="s", name="sp")
        for hl in range(2):
            row = 64 * (pair % 2) + 32 * hl
            nc.tensor.matmul(
                sp[:, hl, :],
                lhsT=s["kt"][row:row + 32, g, 128 * j:128 * (j + 1)],
                rhs=s["qt"][row:row + 32, g, :],
                start=True, stop=True,
                tile_position=(row, 0))
        s["s"][step] = sp

    def exp_step(w, step):
        s = st[w]
        pT = ppool.tile([P, 2, N], adt, tag="pT", name="pT")
        if variant == "exphalf":
            # probe: half the ACT work (other half of pT left stale)
            nc.scalar.activation(pT[:, 0, :], s["s"].pop(step)[:, 0, :],
                                 EXP, scale=SCALE)
        else:
            nc.scalar.activation(pT, s["s"].pop(step), EXP, scale=SCALE)
        s["p"][step] = pT

    def finish_gpair(w, g):
        # pvm: bankpair [128, 2 pairs, 512]: rows [pv_e|d_e|pv_o|d_o] per
        # pair. Stage the d rows in SBUF (approx recip mis-streams on
        # PSUM), reciprocal once per row-block across both pairs, then 4
        # muls into outT.
        s = st[w]
        bankpair = s["pv"].pop(g)
        sbd = mpool.tile([P, N], F32, tag="sbd", name="sbd")
        rc = mpool.tile([P, N], F32, tag="rc", name="rc")
        dcopy = nc.scalar.copy if sched == "pvmbs" else nc.vector.tensor_copy
        for b in range(2):
            for eo in range(2):
                hh = 2 * b + eo
                dcopy(
                    sbd[32 * hh:32 * hh + 32],
                    bankpair[64 * eo + 32:64 * eo + 64, b, :])
        nc.vector.reciprocal_approx_fast(rc, sbd)
        for b in range(2):
            for eo in range(2):
                hh = 2 * b + eo
                nc.vector.tensor_mul(
                    s["outT"][32 * hh:32 * hh + 32, g, :],
                    bankpair[64 * eo:64 * eo + 32, b, :],
                    rc[32 * hh:32 * hh + 32])

    def pvd_step(w, step):
        # col-packed: 2 PV matmuls + 2 denominator matmuls, accum over j
        s = st[w]
        if variant == "nopvd":
            s["p"].pop(step)
            return
        pair, j = pj_of(step)
        if pvm:
            g = pair // 2
            if j == 0 and pair % 2 == 0:
                s["pv"][g] = ps1.tile([P, 2, N], F32, tag="bank",
                                      name="bank")
            bank = s["pv"][g][:, pair % 2, :]
            pT = s["p"].pop(step)
            for eo in range(2):
                nc.tensor.matmul(
                    bank[64 * eo:64 * eo + 64, :],
                    lhsT=s["vv"][:, j, pair, eo, :],
                    rhs=pT[:, eo, :],
                    start=(j == 0), stop=(j == 3),
                    tile_position=(0, 64 * eo), skip_group_check=True)
            if j == 3 and pair % 2 == 1:
                finish_gpair(w, g)
            return
        g = pair // 2
        if step % 8 == 0:
            s["pv"][g] = ps1.tile([P, N], F32, tag="b1", name="pv")
            s["d"][g] = ps1.tile([P, N], F32, tag="b1", name="d")
        pT = s["p"].pop(step)
        for hl in range(2):
            h = 2 * pair + hl
            hh = h % 4
            nc.tensor.matmul(
                s["pv"][g][32 * hh:32 * hh + 32, :],
                lhsT=s["vv"][:, j // 2, (j % 2) * C + 32 * h:(j % 2) * C + 32 * h + 32],
                rhs=pT[:, hl, :],
                start=(j == 0), stop=(j == 3),
                tile_position=(0, 32 * hh), skip_group_check=True)
            if variant == "nodenom" and j > 0:
                continue   # probe: 1 denom matmul per (head) instead of 4
            nc.tensor.matmul(
                s["d"][g][cd_of(hh):cd_of(hh) + 32, :],
                lhsT=ones,
                rhs=pT[:, hl, :],
                start=(j == 0), stop=(j == 3 or variant == "nodenom"),
                tile_position=(0, cd_of(hh)), skip_group_check=True)

    def finish_g(w, g):
        # 1/denom, un-rotating the +64-partition shift the D col-groups use
        # (D lands at (32*hh+64)%128 so it can pack with PV in the PE array)
        if variant == "nopvd" or pvm:
            return
        s = st[w]
        d = s["d"].pop(g)
        rc = mpool.tile([P, N], F32, tag="rc", name="rc")
        sbd = mpool.tile([P, N], F32, tag="sbd", name="sbd")
        # the approx reciprocal only streams correctly on full-128-
        # partition SBUF operands: un-rotate d during the PSUM->SBUF
        # copies, then one full-width approx
        nc.vector.tensor_copy(sbd[0:64], d[64:128])
        nc.vector.tensor_copy(sbd[64:128], d[0:64])
        nc.vector.reciprocal_approx_fast(rc, sbd)
        nc.vector.tensor_mul(s["outT"][:, g, :], s["pv"].pop(g), rc)

    def tail(w):
        s = st[w]
        y = ypool.tile([P, 4, C], F32, tag="y")
        if variant == "notail":
            nc.vector.memset(y[:, 0, :], 0.0)
            nc.sync.dma_start(out_d[w].rearrange("(m p) c -> p m c", p=P), y)
            st[w] = None
            return
        for half in range(2):
            if pvm:
                acc = ps1.tile([P, 2, 2, C], F32, tag="bank",
                               name="acc")[:, 0]
            else:
                acc = acc1()
            for sub in range(2):
                m = 2 * half + sub
                dst = acc[:, sub] if pvm else acc[:, sub * C:(sub + 1) * C]
                for g in range(2):
                    nc.tensor.matmul(
                        dst,
                        lhsT=s["outT"][:, g, 128 * m:128 * (m + 1)],
                        rhs=wproj[:, g, :],
                        start=(g == 0), stop=(g == 1))
            if pvm:
                nc.vector.tensor_add(y[:, 2 * half:2 * half + 2, :],
                                     acc, bias)
            else:
                for sub in range(2):
                    nc.vector.tensor_add(y[:, 2 * half + sub, :],
                                         acc[:, sub * C:(sub + 1) * C],
                                         bias[:, 0, :])
        nc.sync.dma_start(out_d[w].rearrange("(m p) c -> p m c", p=P), y)
        st[w] = None

    # ---- pipelined emission ----
    def one_pass_grouped():
        # Same dataflow as one_pass, but PE matmuls are emitted in
        # same-tiling-mode batches (row-tiled S pairs, then col-tiled PVD
        # pairs) to halve PE array mode switches (each switch drains the
        # array) and put packed matmuls back-to-back so tile-position
        # concurrency can engage.
        start_window(0)
        qkv_qk(0, 0)
        qkv_qk(0, 1)
        qkv_v(0)
        s_step(0, 0)
        s_step(0, 1)
        for w in range(nwin):
            for Dd in range(8):
                t = 2 * Dd
                exp_step(w, t)
                exp_step(w, t + 1)
                for u in (t + 2, t + 3):
                    if u < 16:
                        s_step(w, u)
                    elif w + 1 < nwin:
                        s_step(w + 1, u - 16)
                pvd_step(w, t)
                pvd_step(w, t + 1)
                if Dd == 3:
                    finish_g(w, 0)
                    if w + 1 < nwin:
                        start_window(w + 1)
                        qkv_qk(w + 1, 0)
            finish_g(w, 1)
            if w + 1 < nwin:
                qkv_v(w + 1)
                qkv_qk(w + 1, 1)
            tail(w)

    def s_only_pass(four):
        # probe: ONLY the S matmuls, 2-way vs 4-way row-packed, nothing
        # else on PE. Discriminates serial vs concurrent packed matmuls.
        for w in range(nwin):
            qt = qkpool.tile([P, 2, N], adt, tag="qt", name="qt")
            kt = qkpool.tile([P, 2, N], adt, tag="kt", name="kt")
            nc.vector.memset(qt, 0.1)
            nc.vector.memset(kt, 0.1)
            if four:
                for g in range(2):
                    for j in range(4):
                        spA = ps2.tile([P, 2, N], F32, tag="s", name="spA")
                        spB = ps2.tile([P, 2, N], F32, tag="s", name="spB")
                        for hh in range(4):
                            sp = spA if hh < 2 else spB
                            nc.tensor.matmul(
                                sp[:, hh % 2, :],
                                lhsT=kt[32 * hh:32 * hh + 32, g,
                                        128 * j:128 * (j + 1)],
                                rhs=qt[32 * hh:32 * hh + 32, g, :],
                                start=True, stop=True,
                                tile_position=(32 * hh, 0))
            else:
                for step in range(16):
                    pair, j = step // 4, step % 4
                    g = pair // 2
                    sp = ps2.tile([P, 2, N], F32, tag="s", name="sp")
                    for hl in range(2):
                        row = 64 * (pair % 2) + 32 * hl
                        nc.tensor.matmul(
                            sp[:, hl, :],
                            lhsT=kt[row:row + 32, g, 128 * j:128 * (j + 1)],
                            rhs=qt[row:row + 32, g, :],
                            start=True, stop=True,
                            tile_position=(row, 0))
        y = ypool.tile([P, 4, C], F32, tag="y")
        nc.vector.memset(y, 0.0)
        for w in range(nwin):
            nc.sync.dma_start(out_d[w].rearrange("(m p) c -> p m c", p=P), y)

    def act_only_pass():
        # probe: pure ScalarE stream — 16 activations/window off a fixed
        # PSUM pair, no matmuls. Calibrates peak ACT throughput on HW.
        # variant "actmm" adds an independent PE matmul stream into other
        # PSUM banks to measure ACT/PE PSUM-port contention.
        spA = ps2.tile([P, 2, N], F32, tag="s", name="spA")
        spB = ps2.tile([P, 2, N], F32, tag="s", name="spB")
        nc.vector.memset(spA, 0.25)
        nc.vector.memset(spB, 0.5)
        if variant == "actmm":
            qt = qkpool.tile([P, 2, N], adt, tag="qt", name="qt")
            kt = qkpool.tile([P, 2, N], adt, tag="kt", name="kt")
            nc.vector.memset(qt, 0.1)
            nc.vector.memset(kt, 0.1)
        for w in range(nwin):
            for step in range(16):
                pT = ppool.tile([P, 2, N], adt, tag="pT", name="pT")
                nc.scalar.activation(pT, spA if step % 2 == 0 else spB,
                                     EXP, scale=SCALE)
                if variant == "actmm":
                    mmout = ps1.tile([P, N], F32, tag="b1", name="mmout")
                    for hl in range(2):
                        nc.tensor.matmul(
                            mmout, lhsT=kt[32 * hl:32 * hl + 32, 0, 0:128],
                            rhs=qt[32 * hl:32 * hl + 32, 0, :],
                            start=(hl == 0), stop=(hl == 1),
                            tile_position=(32 * hl, 0),
                            skip_group_check=True)
        # keep the output tensor written so the harness contract holds
        y = ypool.tile([P, 4, C], F32, tag="y")
        nc.vector.memset(y, 0.0)
        for w in range(nwin):
            nc.sync.dma_start(out_d[w].rearrange("(m p) c -> p m c", p=P), y)

    def one_pass():
        start_window(0)
        qkv_qk(0, 0)
        qkv_qk(0, 1)
        qkv_v(0)
        s_step(0, 0)
        s_step(0, 1)
        for w in range(nwin):
            for step in range(16):
                exp_step(w, step)
                t = step + 2
                if sched in ("pvmr", "pvmor"):
                    # PE queue: pvd (longer downstream chain) ahead of
                    # the next s-burst; both gate on exp(step)
                    pvd_step(w, step)
                if t < 16:
                    s_step(w, t)
                elif w + 1 < nwin:
                    s_step(w + 1, t - 16)
                if sched not in ("pvmr", "pvmor"):
                    pvd_step(w, step)
                if step == 7:
                    finish_g(w, 0)
                    if w + 1 < nwin:
                        start_window(w + 1)
                        qkv_qk(w + 1, 0)
            finish_g(w, 1)
            if w + 1 < nwin:
                qkv_v(w + 1)
                qkv_qk(w + 1, 1)
            tail(w)

    if variant in ("actonly", "actmm"):
        body = act_only_pass
    elif variant in ("sonly", "sonly4"):
        def body():
            s_only_pass(variant == "sonly4")
    elif sched in ("grouped", "gpvm", "g4"):
        body = one_pass_grouped
    else:
        body = one_pass
    if reps == 1:
        body()
    else:
        # device-side repeat loop, for timing: isolates kernel time from
        # the ~90ms axon dispatch overhead
        with tc.For_i(0, reps, 1):
            body()


def _build_bass(nwin: int, adt, qdt, reps: int = 1, variant: str = "",
                sched: str = "base") -> bass.Bass:
    nc = bacc.Bacc("TRN2", target_bir_lowering=False)
    x_d = nc.declare_dram_parameter("xt", [nwin, C, N], qdt, isOutput=False)
    wqkv_d = nc.declare_dram_parameter("qkv_w", [C, 3 * C], qdt, isOutput=False)
    wproj_d = nc.declare_dram_parameter("proj_w", [C, C], qdt, isOutput=False)
    bias_d = nc.declare_dram_parameter("proj_b", [C], F32, isOutput=False)
    out_d = nc.declare_dram_parameter("out", [nwin, N, C], F32, isOutput=True)
    from contextlib import ExitStack
    with tile.TileContext(nc) as tc, ExitStack() as ctx:
        _emit(ctx, tc, nwin, adt, qdt, x_d.ap(), wqkv_d.ap(), wproj_d.ap(),
              bias_d.ap(), out_d.ap(), reps=reps, variant=variant,
              sched=sched)
    nc.compile()
    return nc


_CACHE: dict = {}
DTS = {"bf16": BF16, "f32r": F32R, "f32": F32, "f16": F16}

DEFAULT_ATTN = "f16"
DEFAULT_QKV = "f16"
DEFAULT_SCHED = "pvmo"


def get_nc(nwin=WPC, attn=None, qkv=None, reps=1, variant="", sched=None):
    attn = attn or DEFAULT_ATTN
    qkv = qkv or DEFAULT_QKV
    sched = sched or DEFAULT_SCHED
    key = (nwin, attn, qkv, reps, variant, sched)
    if key not in _CACHE:
        _CACHE[key] = _build_bass(nwin, DTS[attn], DTS[qkv], reps=reps,
                                  variant=variant, sched=sched)
    return _CACHE[key]


def shard_inputs(x, qkv_w, proj_w, proj_b, wpc=WPC, qkv=None):
    qkv = qkv or DEFAULT_QKV
    npdt = np.float16 if qkv == "f16" else np.float32
    hn, wn = H // WS, W // WS
    xw = np.asarray(x, dtype=np.float32).reshape(B, T, hn, WS, wn, WS, C)
    xw = xw.transpose(0, 2, 4, 1, 3, 5, 6).reshape(NWIN, N, C)
    xT = np.ascontiguousarray(xw.transpose(0, 2, 1).astype(npdt))
    return [
        {
            "xt": xT[i * wpc:(i + 1) * wpc],
            "qkv_w": np.asarray(qkv_w, dtype=npdt),
            "proj_w": np.asarray(proj_w, dtype=npdt),
            "proj_b": np.asarray(proj_b, dtype=np.float32),
        }
        for i in range(NCORES)
    ]


def unshard(results):
    y = np.concatenate([np.asarray(results[i]["out"]) for i in range(NCORES)],
                       axis=0)
    hn, wn = H // WS, W // WS
    y = y.reshape(B, hn, wn, T, WS, WS, C)
    y = y.transpose(0, 3, 1, 4, 2, 5, 6).reshape(B, T, H, W, C)
    return np.ascontiguousarray(y.astype(np.float32))


def kernel(x, qkv_w, proj_w, proj_b):
    nc = get_nc()
    in_maps = shard_inputs(x, qkv_w, proj_w, proj_b)
    res = run_bass_kernel_spmd(nc, in_maps, list(range(NCORES))).results
    return unshard(res)


if __name__ == "__main__":
    rng = np.random.default_rng(0)
    x = rng.standard_normal((B, T, H, W, C), dtype=np.float32)
    qkv_w = (rng.standard_normal((C, 3 * C), dtype=np.float32) * C ** -0.5)
    proj_w = (rng.standard_normal((C, C), dtype=np.float32) * C ** -0.5)
    proj_b = np.zeros((C,), dtype=np.float32)
    y = kernel(x, qkv_w, proj_w, proj_b)
    print(y.shape, y.dtype)

